# revision 16
# baseline (speedup 1.0000x reference)
"""Multi-head causal attention (B=2, S=2048, D=1024, H=16) on 8 Trainium2
NeuronCores.

Sharding: data-parallel over the 2 batches x tensor-parallel over 4 head
groups (4 heads each).  Core c handles batch c//4, heads [4*(c%4), 4*(c%4)+4).
Each core computes its Q/K/V projections from column shards of Wq/Wk/Wv,
runs causal attention for its heads, and applies its row shard of Wo,
producing a partial (D, S) output.  The host sums the 4 partials per batch
and adds the output bias.

All matmul data is bf16 (fp8 anywhere on the value/prob path passes its
full quantization noise to the output, since attention output is a
random-sign average that shrinks as fast as the noise).

Performance structure:
- Scores for the even/odd head of a pair run on PE row tiles (0,0)/(64,0)
  back-to-back, executing concurrently in the array (64-deep contraction).
- V is augmented with 64 constant columns of 1.0 so the PV matmul itself
  produces the softmax denominator replicated across 64 partitions; a
  shift-copy to base partition 0 (custom DVE ops only work there) feeds
  reciprocal_approx_fast and one tensor_mul.
- Causal diagonal blocks are trimmed: scores/exp/PV only touch the query
  range [d, 512) of diagonal key chunks.
- The next block's Q/K/V projection and the previous block's O-projection
  are sliced into 2-4 matmul "fillers" woven between attention chunks, so
  the tensor engine never idles while the scalar engine runs exp (keeps
  the HAM activity monitor from throttling the PE to half rate).
"""

import sys

sys.path.insert(0, "/opt/trn_rl_repo")

import numpy as np

B, S, D, H = 2, 2048, 1024, 16
DK = D // H            # 64 head dim
NCORES = 8
NGROUPS = 4            # head groups (tensor parallel)
NH = H // NGROUPS      # 4 heads per core
DHL = NH * DK          # 256 local head dims per core
P = 128
DC = D // P            # 8 contraction chunks over D
HC = DHL // P          # 2 local head-dim chunks (= head pairs)
SB = 512               # query block (matmul moving free size)
NSB = S // SB          # 4
SCK = S // P           # 16 key chunks

_CACHE = {}


def _build_nc(causal):
    import concourse.bass as bass
    import concourse.bacc as bacc
    import concourse.mybir as mybir
    import concourse.tile as tile
    from contextlib import ExitStack

    f32 = mybir.dt.float32
    bf16 = mybir.dt.bfloat16
    Exp = mybir.ActivationFunctionType.Exp
    is_ge = mybir.AluOpType.is_ge

    nc = bacc.Bacc(None, target_bir_lowering=False, debug=False)

    xq_d = nc.dram_tensor("xq_t", [D, S], bf16, kind="ExternalInput")
    xk_d = nc.dram_tensor("xk_t", [D, S], bf16, kind="ExternalInput")
    xv_d = nc.dram_tensor("xv_t", [D, S], bf16, kind="ExternalInput")
    # weights pre-arranged on host to the exact SBUF layouts
    wq_d = nc.dram_tensor("wq_a", [P, DC * DHL], bf16, kind="ExternalInput")
    wk_d = nc.dram_tensor("wk_a", [P, DC * DHL], bf16, kind="ExternalInput")
    wv_d = nc.dram_tensor("wv_a", [P, DC * DHL], bf16, kind="ExternalInput")
    wo_d = nc.dram_tensor("wo_a", [P, HC * D], bf16, kind="ExternalInput")
    bq_d = nc.dram_tensor("bq_a", [P, HC], f32, kind="ExternalInput")
    bk_d = nc.dram_tensor("bk_a", [P, HC], f32, kind="ExternalInput")
    bv_d = nc.dram_tensor("bv_a", [1, DHL], f32, kind="ExternalInput")
    out_d = nc.dram_tensor("out_t", [D, S], bf16, kind="ExternalOutput")

    inv_sqrt_dk = 1.0 / float(np.sqrt(DK))

    with tile.TileContext(nc) as tc, ExitStack() as ctx:
        consts = ctx.enter_context(tc.tile_pool(name="consts", bufs=1))
        xpool = ctx.enter_context(tc.tile_pool(name="xpool", bufs=30))
        exp_pool = ctx.enter_context(tc.tile_pool(name="exp_pool", bufs=3))
        small = ctx.enter_context(tc.tile_pool(name="small", bufs=4))
        opool = ctx.enter_context(tc.tile_pool(name="opool", bufs=4))
        proj_ps = ctx.enter_context(
            tc.tile_pool(name="proj_ps", bufs=2, space="PSUM"))
        sc_ps_pool = ctx.enter_context(
            tc.tile_pool(name="sc_ps", bufs=2, space="PSUM"))
        pv_ps_pool = ctx.enter_context(
            tc.tile_pool(name="pv_ps", bufs=2, space="PSUM"))

        # --- resident tensors ---
        wq_sb = consts.tile([P, DC, DHL], bf16)
        wk_sb = consts.tile([P, DC, DHL], bf16)
        wv_sb = consts.tile([P, DC, DHL], bf16)
        wo_sb = consts.tile([P, HC, D], bf16)
        bq_sb = consts.tile([P, HC], f32)
        bk_sb = consts.tile([P, HC], f32)
        bv_row = consts.tile([1, DHL], f32)
        bv_bc = consts.tile([P, DHL], f32)
        q_sb = consts.tile([P, HC, S], bf16)
        k_sb = consts.tile([P, HC, S], bf16)
        # v augmented with 64 constant columns producing the denominator
        v_aug = consts.tile([P, SCK, NH, P], bf16)
        attn_sb = consts.tile([P, HC, S], bf16)
        ones_blk = consts.tile([P, NH, DK], bf16)
        dummy = consts.tile([1, 2], f32)

        # weights split into 2-chunk DMAs so transfers parallelize across
        # DMA engines (a single 512KB descriptor serializes ~23us)
        for w_sb, w_d in ((wq_sb, wq_d), (wk_sb, wk_d), (wv_sb, wv_d)):
            w_r = w_d[:].rearrange("p (c h) -> p c h", c=DC)
            for dc0 in range(0, DC, 2):
                nc.sync.dma_start(w_sb[:, dc0:dc0 + 2, :], w_r[:, dc0:dc0 + 2, :])
        wo_r = wo_d[:].rearrange("p (c o) -> p c o", c=HC)
        for hc in range(HC):
            nc.sync.dma_start(wo_sb[:, hc, :], wo_r[:, hc, :])
        nc.sync.dma_start(bq_sb[:], bq_d[:])
        nc.sync.dma_start(bk_sb[:], bk_d[:])
        nc.sync.dma_start(bv_row[:], bv_d[:])
        nc.gpsimd.partition_broadcast(bv_bc[:], bv_row[:])
        # preload the exp activation table while projections run
        nc.gpsimd.memset(dummy[:], 0.0)
        nc.scalar.activation(dummy[:], dummy[:], Exp, bias=0.0, scale=1.0)
        nc.gpsimd.memset(ones_blk[:], 1.0)
        for sc in range(SCK):
            nc.vector.tensor_copy(v_aug[:, sc, :, DK:P], ones_blk[:])

        bv_bc_r = bv_bc[:].rearrange("p (h e) -> p h e", h=NH)
        xq_r = xq_d[:].rearrange("(c p) s -> p c s", p=P)
        xk_r = xk_d[:].rearrange("(c p) s -> p c s", p=P)
        xv_r = xv_d[:].rearrange("(c p) s -> p c s", p=P)
        out_r = out_d[:].rearrange("(c p) s -> p c s", p=P)

        xts = {}

        def emit_x_dmas(sbi):
            ss = slice(sbi * SB, (sbi + 1) * SB)
            for name, x_r in (("q", xq_r), ("k", xk_r), ("v", xv_r)):
                for dc in range(DC):
                    x_tile = xpool.tile([P, SB], bf16,
                                        name=f"xt{name}{sbi}_{dc}", tag="xs")
                    nc.sync.dma_start(x_tile[:], x_r[:, dc, ss])
                    xts[(name, sbi, dc)] = x_tile

        def qk_proj_slices(sbi, name, w_sb, b_sb, t_sb):
            """Yield filler closures: one q/k projection unit (hc) in 4
            slices of 2 matmuls, plus the bias-add."""
            ss = slice(sbi * SB, (sbi + 1) * SB)
            for hc in range(HC):
                cell = {}

                def mk(cell, hc, dc0):
                    def emit():
                        if dc0 == 0:
                            cell["ps"] = proj_ps.tile([P, SB], f32,
                                                      name="proj", tag="proj")
                        ps = cell["ps"]
                        for dc in (dc0, dc0 + 1):
                            nc.tensor.matmul(
                                ps[:], w_sb[:, dc, hc * P:(hc + 1) * P],
                                xts[(name, sbi, dc)][:], start=(dc == 0),
                                stop=(dc == DC - 1))
                        if dc0 + 2 == DC:
                            nc.vector.tensor_scalar_add(
                                t_sb[:, hc, ss], ps[:], b_sb[:, hc:hc + 1])
                    return emit
                for dc0 in range(0, DC, 2):
                    yield mk(cell, hc, dc0)

        def v_proj_slices(sbi):
            """Yield filler closures: one v projection unit (scl) in 2
            slices of 4 matmuls, plus the bias-add."""
            for scl in range(SB // P):
                sc = sbi * (SB // P) + scl
                cell = {}

                def mk(cell, sc, scl, dc0):
                    def emit():
                        if dc0 == 0:
                            cell["ps"] = proj_ps.tile([P, SB], f32,
                                                      name="proj", tag="proj")
                        ps = cell["ps"]
                        for dc in range(dc0, dc0 + 4):
                            nc.tensor.matmul(
                                ps[:, 0:DHL],
                                xts[("v", sbi, dc)][:, scl * P:(scl + 1) * P],
                                wv_sb[:, dc, :], start=(dc == 0),
                                stop=(dc == DC - 1))
                        if dc0 + 4 == DC:
                            nc.vector.tensor_add(
                                v_aug[:, sc, :, 0:DK],
                                ps[:, 0:DHL].rearrange(
                                    "p (h e) -> p h e", h=NH), bv_bc_r)
                    return emit
                for dc0 in range(0, DC, 4):
                    yield mk(cell, sc, scl, dc0)

        def proj_slices(sbi):
            yield from qk_proj_slices(sbi, "q", wq_sb, bq_sb, q_sb)
            yield from qk_proj_slices(sbi, "k", wk_sb, bk_sb, k_sb)
            yield from v_proj_slices(sbi)

        def oproj_slices(qb, split_dma=1):
            """Yield filler closures: output projection, one oc per slice."""
            ss = slice(qb * SB, (qb + 1) * SB)
            for oc in range(DC):
                def mk(oc):
                    def emit():
                        ps = proj_ps.tile([P, SB], f32, name="proj",
                                          tag="proj")
                        for hc in range(HC):
                            nc.tensor.matmul(
                                ps[:], wo_sb[:, hc, oc * P:(oc + 1) * P],
                                attn_sb[:, hc, ss], start=(hc == 0),
                                stop=(hc == HC - 1))
                        o_tile = opool.tile([P, SB], bf16, name="ot", tag="ot")
                        nc.vector.tensor_copy(o_tile[:], ps[:])
                        w = SB // split_dma
                        for si in range(split_dma):
                            nc.sync.dma_start(
                                out_r[:, oc, qb * SB + si * w:
                                      qb * SB + (si + 1) * w],
                                o_tile[:, si * w:(si + 1) * w])
                    return emit
                yield mk(oc)

        # ---- prologue: x(0), dense projection of block 0, then x(1) ----
        emit_x_dmas(0)
        for emit in proj_slices(0):
            emit()
        emit_x_dmas(1)

        # filler distribution: O-projections are pushed as late as possible
        # so the final (largest) attention block still has tensor work to
        # absorb exp latency (keeps HAM from throttling the PE)
        filler_plan = {
            0: [("proj", 1)],
            1: [("proj", 2)],
            2: [("oproj", 0), ("proj", 3)],
            3: [("oproj", 1), ("oproj", 2)],
        }
        for sbi in range(NSB):
            if sbi + 2 < NSB:
                emit_x_dmas(sbi + 2)
            fillers = []
            for kind, idx in filler_plan[sbi]:
                if kind == "proj":
                    fillers.extend(proj_slices(idx))
                else:
                    fillers.extend(oproj_slices(idx))
            fillers.reverse()  # pop() from the front order

            # ---- attention for query block qb = sbi, head pairs ----
            qb = sbi
            ss = slice(qb * SB, (qb + 1) * SB)
            n_chunks = (qb + 1) * (SB // P) if causal else SCK
            for pair in range(HC):
                pv0 = pv_ps_pool.tile([P, SB], f32, name="pv0", tag="pv")
                pv1 = pv_ps_pool.tile([P, SB], f32, name="pv1", tag="pv")
                exs = [None, None]
                for tj in range(n_chunks):
                    # diagonal trim: queries [qlo, SB) of this key chunk
                    d = tj * P - qb * SB
                    qlo = max(0, d) if causal else 0
                    qs = slice(qb * SB + qlo, (qb + 1) * SB)
                    sc_t = sc_ps_pool.tile([P, 2, SB], f32, name="sc",
                                           tag="sc")
                    ko = tj * P
                    # even/odd head on PE row-tiles 0/64: concurrent
                    nc.tensor.matmul(
                        sc_t[:, 0, qlo:SB], k_sb[0:DK, pair, ko:ko + P],
                        q_sb[0:DK, pair, qs], start=True, stop=True)
                    nc.tensor.matmul(
                        sc_t[:, 1, qlo:SB], k_sb[DK:P, pair, ko:ko + P],
                        q_sb[DK:P, pair, qs], start=True, stop=True)
                    ex = exp_pool.tile([P, 2, SB], bf16, name="ex", tag="ex")
                    nc.scalar.activation(ex[:, :, qlo:SB], sc_t[:, :, qlo:SB],
                                         Exp, bias=0.0, scale=inv_sqrt_dk)
                    if causal and d > -P:
                        # partial rows: keep element when q' - p >= -d + qlo
                        nc.gpsimd.affine_select(
                            ex[:, :, qlo:SB], ex[:, :, qlo:SB],
                            pattern=[[0, 2], [1, SB - qlo]], compare_op=is_ge,
                            fill=0.0, base=qlo - d, channel_multiplier=-1)
                    # software pipeline: PV of chunk tj-1 after scores of tj
                    if tj > 0:
                        pex, pqlo = exs[(tj - 1) % 2]
                        nc.tensor.matmul(
                            pv0[:, pqlo:SB], v_aug[:, tj - 1, 2 * pair, :],
                            pex[:, 0, pqlo:SB], start=(tj == 1), stop=False)
                        nc.tensor.matmul(
                            pv1[:, pqlo:SB], v_aug[:, tj - 1, 2 * pair + 1, :],
                            pex[:, 1, pqlo:SB], start=(tj == 1), stop=False)
                    exs[tj % 2] = (ex, qlo)
                    # keep the tensor queue dense while exp runs
                    for _ in range(2):
                        if fillers:
                            fillers.pop()()
                tj = n_chunks - 1
                pex, pqlo = exs[tj % 2]
                nc.tensor.matmul(
                    pv0[:, pqlo:SB], v_aug[:, tj, 2 * pair, :],
                    pex[:, 0, pqlo:SB], start=(n_chunks == 1), stop=True)
                nc.tensor.matmul(
                    pv1[:, pqlo:SB], v_aug[:, tj, 2 * pair + 1, :],
                    pex[:, 1, pqlo:SB], start=(n_chunks == 1), stop=True)
                for j2, pv in ((0, pv0), (1, pv1)):
                    po = j2 * DK
                    den = small.tile([DK, SB], f32, name=f"dn{j2}", tag="den")
                    # custom DVE ops require base partition 0: shift-copy the
                    # denominator rows down before the fast reciprocal
                    nc.vector.tensor_copy(den[:], pv[DK:P, :])
                    recip = small.tile([DK, SB], f32, name=f"rc{j2}",
                                       tag="recip")
                    nc.vector.reciprocal_approx_fast(recip[:], den[:])
                    nc.vector.tensor_mul(
                        attn_sb[po:po + DK, pair, ss], pv[0:DK, :], recip[:])
            while fillers:
                fillers.pop()()

        for emit in oproj_slices(NSB - 1, split_dma=2):
            emit()

    nc.compile()
    return nc


def _get_nc(causal):
    key = ("causal" if causal else "dense")
    if key not in _CACHE:
        _CACHE[key] = _build_nc(causal)
    return _CACHE[key]


def _prep_core_inputs(Q, K, V, Wq, bq, Wk, bk, Wv, bv, Wo):
    """Build the 8 per-core input maps (bf16 data, baseline layouts)."""
    import ml_dtypes
    bf16 = ml_dtypes.bfloat16
    cc = np.ascontiguousarray

    in_maps = []
    for c in range(NCORES):
        b = c // NGROUPS
        g = c % NGROUPS
        hs, he = g * DHL, (g + 1) * DHL
        # weights pre-arranged to SBUF layout [128, DC, DHL] with d = dc*128+p
        wq_a = cc(Wq[hs:he, :].T.reshape(DC, P, DHL).transpose(1, 0, 2)
                  .reshape(P, DC * DHL))
        wk_a = cc(Wk[hs:he, :].T.reshape(DC, P, DHL).transpose(1, 0, 2)
                  .reshape(P, DC * DHL))
        wv_a = cc(Wv[hs:he, :].T.reshape(DC, P, DHL).transpose(1, 0, 2)
                  .reshape(P, DC * DHL))
        # Wo shard: lhsT layout [hd, dout] split to [128, HC, D], hd = hc*128+p
        wo_a = cc(Wo[:, hs:he].T.reshape(HC, P, D).transpose(1, 0, 2)
                  .reshape(P, HC * D))
        in_maps.append({
            "xq_t": cc(Q[b].T).astype(bf16), "xk_t": cc(K[b].T).astype(bf16),
            "xv_t": cc(V[b].T).astype(bf16),
            "wq_a": wq_a.astype(bf16), "wk_a": wk_a.astype(bf16),
            "wv_a": wv_a.astype(bf16), "wo_a": wo_a.astype(bf16),
            "bq_a": cc(bq[hs:he].reshape(HC, P).T),
            "bk_a": cc(bk[hs:he].reshape(HC, P).T),
            "bv_a": cc(bv[hs:he].reshape(1, DHL)),
        })
    return in_maps


def _classify_mask(mask):
    m = np.asarray(mask)
    if m.dtype != np.bool_:
        m = m.astype(bool)
    causal = np.tril(np.ones((S, S), dtype=bool))
    if all(np.array_equal(m[b, 0], causal) for b in range(m.shape[0])):
        return "causal"
    if m.all():
        return "dense"
    return "generic"


def _numpy_reference(Q, K, V, mask, Wq, bq, Wk, bk, Wv, bv, Wo, bo):
    """Plain numpy fallback for arbitrary masks."""
    out = np.empty((B, S, D), dtype=np.float32)
    for b in range(B):
        q = (Q[b] @ Wq.T + bq).reshape(S, H, DK).transpose(1, 0, 2)
        k = (K[b] @ Wk.T + bk).reshape(S, H, DK).transpose(1, 0, 2)
        v = (V[b] @ Wv.T + bv).reshape(S, H, DK).transpose(1, 0, 2)
        m = np.asarray(mask[b, 0], dtype=bool)
        acc = np.empty((H, S, DK), dtype=np.float32)
        for h in range(H):
            s = (q[h] @ k[h].T) / np.float32(np.sqrt(DK))
            s = np.where(m, s, np.float32(-1e9))
            s = s - s.max(axis=-1, keepdims=True)
            e = np.exp(s)
            p = e / e.sum(axis=-1, keepdims=True)
            acc[h] = p @ v[h]
        out[b] = acc.transpose(1, 0, 2).reshape(S, D) @ Wo.T + bo
    return out


def kernel(Q, K, V, mask, Wq, bq, Wk, bk, Wv, bv, Wo, bo,
           _profile=False, _trace_dir=None):
    from concourse.bass_utils import run_bass_kernel_spmd

    flavor = _classify_mask(mask)
    if flavor == "generic":
        return _numpy_reference(Q, K, V, mask, Wq, bq, Wk, bk, Wv, bv, Wo, bo)

    nc = _get_nc(flavor == "causal")
    in_maps = _prep_core_inputs(
        np.asarray(Q, np.float32), np.asarray(K, np.float32),
        np.asarray(V, np.float32), np.asarray(Wq, np.float32),
        np.asarray(bq, np.float32), np.asarray(Wk, np.float32),
        np.asarray(bk, np.float32), np.asarray(Wv, np.float32),
        np.asarray(bv, np.float32), np.asarray(Wo, np.float32))

    kwargs = {}
    if _profile:
        import types as _types
        if "antenv.axon_hooks" not in sys.modules:
            sys.path.insert(0, "/root/.axon_site")
            from trn_agent_boot.trn_boot import _ntff_profile_via_ctypes
            _hook = _ntff_profile_via_ctypes("/opt/axon/libaxon_pjrt.so")
            _mod = _types.ModuleType("antenv.axon_hooks")
            _mod.get_axon_ntff_profile_hook = lambda: _hook
            _mod.set_axon_ntff_profile_hook = lambda h: None
            sys.modules["antenv.axon_hooks"] = _mod
        import concourse.bass_utils as _bu
        _bu.upload_artifacts = lambda d: d  # no cloud copy in this container
        kwargs = dict(trace=True, trace_cores=[0])
        if _trace_dir is not None:
            kwargs["tmpdir"] = _trace_dir
    res = run_bass_kernel_spmd(nc, in_maps, core_ids=list(range(NCORES)),
                               **kwargs)

    out = np.empty((B, S, D), dtype=np.float32)
    bo32 = np.asarray(bo, np.float32)
    for b in range(B):
        acc = res.results[b * NGROUPS]["out_t"].astype(np.float32)
        for g in range(1, NGROUPS):
            acc = acc + res.results[b * NGROUPS + g]["out_t"].astype(
                np.float32)
        out[b] = acc.T + bo32
    if _profile:
        kernel._last_exec_time_ns = res.exec_time_ns
        kernel._last_results = res
    return out


# revision 19
# speedup vs baseline: 1.0625x; 1.0625x over previous
"""Multi-head causal attention (B=2, S=2048, D=1024, H=16) on 8 Trainium2
NeuronCores.

Sharding: data-parallel over the 2 batches x tensor-parallel over 4 head
groups (4 heads each).  Core c handles batch c//4, heads [4*(c%4), 4*(c%4)+4).
Each core computes its Q/K/V projections from column shards of Wq/Wk/Wv,
runs causal attention for its heads, and applies its row shard of Wo,
producing a partial (D, S) output.  The host sums the 4 partials per batch
and adds the output bias.

All matmul data is bf16 (fp8 anywhere on the value/prob path passes its
full quantization noise to the output, since attention output is a
random-sign average that shrinks as fast as the noise).

Performance structure:
- Scores for the even/odd head of a pair run on PE row tiles (0,0)/(64,0)
  back-to-back, executing concurrently in the array (64-deep contraction).
- V is augmented with 64 constant columns of 1.0 so the PV matmul itself
  produces the softmax denominator replicated across 64 partitions; a
  shift-copy to base partition 0 (custom DVE ops only work there) feeds
  reciprocal_approx_fast and one tensor_mul.
- Causal diagonal blocks are trimmed: scores/exp/PV only touch the query
  range [d, 512) of diagonal key chunks.
- The next block's Q/K/V projection and the previous block's O-projection
  are sliced into 2-4 matmul "fillers" woven between attention chunks, so
  the tensor engine never idles while the scalar engine runs exp (keeps
  the HAM activity monitor from throttling the PE to half rate).
"""

import sys

sys.path.insert(0, "/opt/trn_rl_repo")

import numpy as np

B, S, D, H = 2, 2048, 1024, 16
DK = D // H            # 64 head dim
NCORES = 8
NGROUPS = 4            # head groups (tensor parallel)
NH = H // NGROUPS      # 4 heads per core
DHL = NH * DK          # 256 local head dims per core
P = 128
DC = D // P            # 8 contraction chunks over D
HC = DHL // P          # 2 local head-dim chunks (= head pairs)
SB = 512               # query block (matmul moving free size)
NSB = S // SB          # 4
SCK = S // P           # 16 key chunks

_CACHE = {}


def _build_nc(causal):
    import concourse.bass as bass
    import concourse.bacc as bacc
    import concourse.mybir as mybir
    import concourse.tile as tile
    from contextlib import ExitStack

    f32 = mybir.dt.float32
    bf16 = mybir.dt.bfloat16
    Exp = mybir.ActivationFunctionType.Exp
    is_ge = mybir.AluOpType.is_ge

    nc = bacc.Bacc(None, target_bir_lowering=False, debug=False)

    xq_d = nc.dram_tensor("xq_t", [D, S], bf16, kind="ExternalInput")
    xk_d = nc.dram_tensor("xk_t", [D, S], bf16, kind="ExternalInput")
    xv_d = nc.dram_tensor("xv_t", [D, S], bf16, kind="ExternalInput")
    # weights pre-arranged on host to the exact SBUF layouts
    wq_d = nc.dram_tensor("wq_a", [P, DC * DHL], bf16, kind="ExternalInput")
    wk_d = nc.dram_tensor("wk_a", [P, DC * DHL], bf16, kind="ExternalInput")
    wv_d = nc.dram_tensor("wv_a", [P, DC * DHL], bf16, kind="ExternalInput")
    wo_d = nc.dram_tensor("wo_a", [P, HC * D], bf16, kind="ExternalInput")
    bq_d = nc.dram_tensor("bq_a", [P, HC], f32, kind="ExternalInput")
    bk_d = nc.dram_tensor("bk_a", [P, HC], f32, kind="ExternalInput")
    bv_d = nc.dram_tensor("bv_a", [1, DHL], f32, kind="ExternalInput")
    out_d = nc.dram_tensor("out_t", [D, S], bf16, kind="ExternalOutput")

    inv_sqrt_dk = 1.0 / float(np.sqrt(DK))

    with tile.TileContext(nc) as tc, ExitStack() as ctx:
        consts = ctx.enter_context(tc.tile_pool(name="consts", bufs=1))
        xpool = ctx.enter_context(tc.tile_pool(name="xpool", bufs=9))
        exp_pool = ctx.enter_context(tc.tile_pool(name="exp_pool", bufs=3))
        small = ctx.enter_context(tc.tile_pool(name="small", bufs=4))
        opool = ctx.enter_context(tc.tile_pool(name="opool", bufs=4))
        proj_ps = ctx.enter_context(
            tc.tile_pool(name="proj_ps", bufs=2, space="PSUM"))
        sc_ps_pool = ctx.enter_context(
            tc.tile_pool(name="sc_ps", bufs=2, space="PSUM"))
        pv_ps_pool = ctx.enter_context(
            tc.tile_pool(name="pv_ps", bufs=2, space="PSUM"))

        # --- resident tensors ---
        wq_sb = consts.tile([P, DC, DHL], bf16)
        wk_sb = consts.tile([P, DC, DHL], bf16)
        wv_sb = consts.tile([P, DC, DHL], bf16)
        wo_sb = consts.tile([P, HC, D], bf16)
        bq_sb = consts.tile([P, HC], f32)
        bk_sb = consts.tile([P, HC], f32)
        bv_row = consts.tile([1, DHL], f32)
        bv_bc = consts.tile([P, DHL], f32)
        q_sb = consts.tile([P, HC, S], bf16)
        k_sb = consts.tile([P, HC, S], bf16)
        # v augmented with 64 constant columns producing the denominator
        v_aug = consts.tile([P, SCK, NH, P], bf16)
        attn_sb = consts.tile([P, HC, S], bf16)
        ones_blk = consts.tile([P, NH, DK], bf16)
        dummy = consts.tile([1, 2], f32)

        # one descriptor per weight tensor: the Sync queue issues descriptors
        # at ~600ns each, so descriptor COUNT (not transfer size) is the
        # startup serializer; packets of one descriptor spread across engines
        nc.sync.dma_start(wq_sb[:], wq_d[:].rearrange("p (c h) -> p c h", c=DC))
        nc.sync.dma_start(wk_sb[:], wk_d[:].rearrange("p (c h) -> p c h", c=DC))
        nc.sync.dma_start(wv_sb[:], wv_d[:].rearrange("p (c h) -> p c h", c=DC))
        nc.sync.dma_start(wo_sb[:], wo_d[:].rearrange("p (c o) -> p c o", c=HC))
        nc.sync.dma_start(bq_sb[:], bq_d[:])
        nc.sync.dma_start(bk_sb[:], bk_d[:])
        nc.sync.dma_start(bv_row[:], bv_d[:])
        nc.gpsimd.partition_broadcast(bv_bc[:], bv_row[:])
        # preload the exp activation table while projections run
        nc.gpsimd.memset(dummy[:], 0.0)
        nc.scalar.activation(dummy[:], dummy[:], Exp, bias=0.0, scale=1.0)
        nc.gpsimd.memset(ones_blk[:], 1.0)
        for sc in range(SCK):
            nc.vector.tensor_copy(v_aug[:, sc, :, DK:P], ones_blk[:])

        bv_bc_r = bv_bc[:].rearrange("p (h e) -> p h e", h=NH)
        xq_r = xq_d[:].rearrange("(c p) s -> p c s", p=P)
        xk_r = xk_d[:].rearrange("(c p) s -> p c s", p=P)
        xv_r = xv_d[:].rearrange("(c p) s -> p c s", p=P)
        out_r = out_d[:].rearrange("(c p) s -> p c s", p=P)

        xts = {}

        def emit_x_dmas(sbi):
            ss = slice(sbi * SB, (sbi + 1) * SB)
            for name, x_r in (("q", xq_r), ("k", xk_r), ("v", xv_r)):
                x_tile = xpool.tile([P, DC, SB], bf16,
                                    name=f"xt{name}{sbi}", tag="xs")
                nc.sync.dma_start(x_tile[:], x_r[:, :, ss])
                xts[(name, sbi)] = x_tile

        def qk_proj_slices(sbi, name, w_sb, b_sb, t_sb):
            """Yield filler closures: one q/k projection unit (hc) in 4
            slices of 2 matmuls, plus the bias-add."""
            ss = slice(sbi * SB, (sbi + 1) * SB)
            for hc in range(HC):
                cell = {}

                def mk(cell, hc, dc0):
                    def emit():
                        if dc0 == 0:
                            cell["ps"] = proj_ps.tile([P, SB], f32,
                                                      name="proj", tag="proj")
                        ps = cell["ps"]
                        for dc in (dc0, dc0 + 1):
                            nc.tensor.matmul(
                                ps[:], w_sb[:, dc, hc * P:(hc + 1) * P],
                                xts[(name, sbi)][:, dc, :], start=(dc == 0),
                                stop=(dc == DC - 1))
                        if dc0 + 2 == DC:
                            nc.vector.tensor_scalar_add(
                                t_sb[:, hc, ss], ps[:], b_sb[:, hc:hc + 1])
                    return emit
                for dc0 in range(0, DC, 2):
                    yield mk(cell, hc, dc0)

        def v_proj_slices(sbi):
            """Yield filler closures: one v projection unit (scl) in 2
            slices of 4 matmuls, plus the bias-add."""
            for scl in range(SB // P):
                sc = sbi * (SB // P) + scl
                cell = {}

                def mk(cell, sc, scl, dc0):
                    def emit():
                        if dc0 == 0:
                            cell["ps"] = proj_ps.tile([P, SB], f32,
                                                      name="proj", tag="proj")
                        ps = cell["ps"]
                        for dc in range(dc0, dc0 + 4):
                            nc.tensor.matmul(
                                ps[:, 0:DHL],
                                xts[("v", sbi)][:, dc, scl * P:(scl + 1) * P],
                                wv_sb[:, dc, :], start=(dc == 0),
                                stop=(dc == DC - 1))
                        if dc0 + 4 == DC:
                            nc.vector.tensor_add(
                                v_aug[:, sc, :, 0:DK],
                                ps[:, 0:DHL].rearrange(
                                    "p (h e) -> p h e", h=NH), bv_bc_r)
                    return emit
                for dc0 in range(0, DC, 4):
                    yield mk(cell, sc, scl, dc0)

        def proj_slices(sbi):
            yield from qk_proj_slices(sbi, "q", wq_sb, bq_sb, q_sb)
            yield from qk_proj_slices(sbi, "k", wk_sb, bk_sb, k_sb)
            yield from v_proj_slices(sbi)

        def oproj_slices(qb, split_dma=1):
            """Yield filler closures: output projection, one oc per slice."""
            ss = slice(qb * SB, (qb + 1) * SB)
            for oc in range(DC):
                def mk(oc):
                    def emit():
                        ps = proj_ps.tile([P, SB], f32, name="proj",
                                          tag="proj")
                        for hc in range(HC):
                            nc.tensor.matmul(
                                ps[:], wo_sb[:, hc, oc * P:(oc + 1) * P],
                                attn_sb[:, hc, ss], start=(hc == 0),
                                stop=(hc == HC - 1))
                        o_tile = opool.tile([P, SB], bf16, name="ot", tag="ot")
                        nc.vector.tensor_copy(o_tile[:], ps[:])
                        w = SB // split_dma
                        for si in range(split_dma):
                            nc.sync.dma_start(
                                out_r[:, oc, qb * SB + si * w:
                                      qb * SB + (si + 1) * w],
                                o_tile[:, si * w:(si + 1) * w])
                    return emit
                yield mk(oc)

        # ---- prologue: x(0), dense projection of block 0, then x(1) ----
        emit_x_dmas(0)
        for emit in proj_slices(0):
            emit()
        emit_x_dmas(1)

        # filler distribution: O-projections are pushed as late as possible
        # so the final (largest) attention block still has tensor work to
        # absorb exp latency (keeps HAM from throttling the PE)
        filler_plan = {
            0: [("proj", 1)],
            1: [("proj", 2)],
            2: [("oproj", 0), ("proj", 3)],
            3: [("oproj", 1), ("oproj", 2)],
        }
        for sbi in range(NSB):
            if sbi + 2 < NSB:
                emit_x_dmas(sbi + 2)
            fillers = []
            for kind, idx in filler_plan[sbi]:
                if kind == "proj":
                    fillers.extend(proj_slices(idx))
                else:
                    fillers.extend(oproj_slices(idx))
            fillers.reverse()  # pop() from the front order

            # ---- attention for query block qb = sbi, head pairs ----
            qb = sbi
            ss = slice(qb * SB, (qb + 1) * SB)
            n_chunks = (qb + 1) * (SB // P) if causal else SCK
            for pair in range(HC):
                pv0 = pv_ps_pool.tile([P, SB], f32, name="pv0", tag="pv")
                pv1 = pv_ps_pool.tile([P, SB], f32, name="pv1", tag="pv")
                exs = [None, None]
                for tj in range(n_chunks):
                    # diagonal trim: queries [qlo, SB) of this key chunk
                    d = tj * P - qb * SB
                    qlo = max(0, d) if causal else 0
                    qs = slice(qb * SB + qlo, (qb + 1) * SB)
                    sc_t = sc_ps_pool.tile([P, 2, SB], f32, name="sc",
                                           tag="sc")
                    ko = tj * P
                    # even/odd head on PE row-tiles 0/64: concurrent
                    nc.tensor.matmul(
                        sc_t[:, 0, qlo:SB], k_sb[0:DK, pair, ko:ko + P],
                        q_sb[0:DK, pair, qs], start=True, stop=True)
                    nc.tensor.matmul(
                        sc_t[:, 1, qlo:SB], k_sb[DK:P, pair, ko:ko + P],
                        q_sb[DK:P, pair, qs], start=True, stop=True)
                    ex = exp_pool.tile([P, 2, SB], bf16, name="ex", tag="ex")
                    nc.scalar.activation(ex[:, :, qlo:SB], sc_t[:, :, qlo:SB],
                                         Exp, bias=0.0, scale=inv_sqrt_dk)
                    if causal and d > -P:
                        # partial rows: keep element when q' - p >= -d + qlo
                        nc.gpsimd.affine_select(
                            ex[:, :, qlo:SB], ex[:, :, qlo:SB],
                            pattern=[[0, 2], [1, SB - qlo]], compare_op=is_ge,
                            fill=0.0, base=qlo - d, channel_multiplier=-1)
                    # software pipeline: PV of chunk tj-1 after scores of tj
                    if tj > 0:
                        pex, pqlo = exs[(tj - 1) % 2]
                        nc.tensor.matmul(
                            pv0[:, pqlo:SB], v_aug[:, tj - 1, 2 * pair, :],
                            pex[:, 0, pqlo:SB], start=(tj == 1), stop=False)
                        nc.tensor.matmul(
                            pv1[:, pqlo:SB], v_aug[:, tj - 1, 2 * pair + 1, :],
                            pex[:, 1, pqlo:SB], start=(tj == 1), stop=False)
                    exs[tj % 2] = (ex, qlo)
                    # keep the tensor queue dense while exp runs
                    for _ in range(2):
                        if fillers:
                            fillers.pop()()
                tj = n_chunks - 1
                pex, pqlo = exs[tj % 2]
                nc.tensor.matmul(
                    pv0[:, pqlo:SB], v_aug[:, tj, 2 * pair, :],
                    pex[:, 0, pqlo:SB], start=(n_chunks == 1), stop=True)
                nc.tensor.matmul(
                    pv1[:, pqlo:SB], v_aug[:, tj, 2 * pair + 1, :],
                    pex[:, 1, pqlo:SB], start=(n_chunks == 1), stop=True)
                for j2, pv in ((0, pv0), (1, pv1)):
                    po = j2 * DK
                    den = small.tile([DK, SB], f32, name=f"dn{j2}", tag="den")
                    # custom DVE ops require base partition 0: shift-copy the
                    # denominator rows down before the fast reciprocal
                    nc.vector.tensor_copy(den[:], pv[DK:P, :])
                    recip = small.tile([DK, SB], f32, name=f"rc{j2}",
                                       tag="recip")
                    nc.vector.reciprocal_approx_fast(recip[:], den[:])
                    nc.vector.tensor_mul(
                        attn_sb[po:po + DK, pair, ss], pv[0:DK, :], recip[:])
            while fillers:
                fillers.pop()()

        for emit in oproj_slices(NSB - 1):
            emit()

    nc.compile()
    return nc


def _get_nc(causal):
    key = ("causal" if causal else "dense")
    if key not in _CACHE:
        _CACHE[key] = _build_nc(causal)
    return _CACHE[key]


def _prep_core_inputs(Q, K, V, Wq, bq, Wk, bk, Wv, bv, Wo):
    """Build the 8 per-core input maps (bf16 data, baseline layouts)."""
    import ml_dtypes
    bf16 = ml_dtypes.bfloat16
    cc = np.ascontiguousarray

    in_maps = []
    for c in range(NCORES):
        b = c // NGROUPS
        g = c % NGROUPS
        hs, he = g * DHL, (g + 1) * DHL
        # weights pre-arranged to SBUF layout [128, DC, DHL] with d = dc*128+p
        wq_a = cc(Wq[hs:he, :].T.reshape(DC, P, DHL).transpose(1, 0, 2)
                  .reshape(P, DC * DHL))
        wk_a = cc(Wk[hs:he, :].T.reshape(DC, P, DHL).transpose(1, 0, 2)
                  .reshape(P, DC * DHL))
        wv_a = cc(Wv[hs:he, :].T.reshape(DC, P, DHL).transpose(1, 0, 2)
                  .reshape(P, DC * DHL))
        # Wo shard: lhsT layout [hd, dout] split to [128, HC, D], hd = hc*128+p
        wo_a = cc(Wo[:, hs:he].T.reshape(HC, P, D).transpose(1, 0, 2)
                  .reshape(P, HC * D))
        in_maps.append({
            "xq_t": cc(Q[b].T).astype(bf16), "xk_t": cc(K[b].T).astype(bf16),
            "xv_t": cc(V[b].T).astype(bf16),
            "wq_a": wq_a.astype(bf16), "wk_a": wk_a.astype(bf16),
            "wv_a": wv_a.astype(bf16), "wo_a": wo_a.astype(bf16),
            "bq_a": cc(bq[hs:he].reshape(HC, P).T),
            "bk_a": cc(bk[hs:he].reshape(HC, P).T),
            "bv_a": cc(bv[hs:he].reshape(1, DHL)),
        })
    return in_maps


def _classify_mask(mask):
    m = np.asarray(mask)
    if m.dtype != np.bool_:
        m = m.astype(bool)
    causal = np.tril(np.ones((S, S), dtype=bool))
    if all(np.array_equal(m[b, 0], causal) for b in range(m.shape[0])):
        return "causal"
    if m.all():
        return "dense"
    return "generic"


def _numpy_reference(Q, K, V, mask, Wq, bq, Wk, bk, Wv, bv, Wo, bo):
    """Plain numpy fallback for arbitrary masks."""
    out = np.empty((B, S, D), dtype=np.float32)
    for b in range(B):
        q = (Q[b] @ Wq.T + bq).reshape(S, H, DK).transpose(1, 0, 2)
        k = (K[b] @ Wk.T + bk).reshape(S, H, DK).transpose(1, 0, 2)
        v = (V[b] @ Wv.T + bv).reshape(S, H, DK).transpose(1, 0, 2)
        m = np.asarray(mask[b, 0], dtype=bool)
        acc = np.empty((H, S, DK), dtype=np.float32)
        for h in range(H):
            s = (q[h] @ k[h].T) / np.float32(np.sqrt(DK))
            s = np.where(m, s, np.float32(-1e9))
            s = s - s.max(axis=-1, keepdims=True)
            e = np.exp(s)
            p = e / e.sum(axis=-1, keepdims=True)
            acc[h] = p @ v[h]
        out[b] = acc.transpose(1, 0, 2).reshape(S, D) @ Wo.T + bo
    return out


def kernel(Q, K, V, mask, Wq, bq, Wk, bk, Wv, bv, Wo, bo,
           _profile=False, _trace_dir=None):
    from concourse.bass_utils import run_bass_kernel_spmd

    flavor = _classify_mask(mask)
    if flavor == "generic":
        return _numpy_reference(Q, K, V, mask, Wq, bq, Wk, bk, Wv, bv, Wo, bo)

    nc = _get_nc(flavor == "causal")
    in_maps = _prep_core_inputs(
        np.asarray(Q, np.float32), np.asarray(K, np.float32),
        np.asarray(V, np.float32), np.asarray(Wq, np.float32),
        np.asarray(bq, np.float32), np.asarray(Wk, np.float32),
        np.asarray(bk, np.float32), np.asarray(Wv, np.float32),
        np.asarray(bv, np.float32), np.asarray(Wo, np.float32))

    kwargs = {}
    if _profile:
        import types as _types
        if "antenv.axon_hooks" not in sys.modules:
            sys.path.insert(0, "/root/.axon_site")
            from trn_agent_boot.trn_boot import _ntff_profile_via_ctypes
            _hook = _ntff_profile_via_ctypes("/opt/axon/libaxon_pjrt.so")
            _mod = _types.ModuleType("antenv.axon_hooks")
            _mod.get_axon_ntff_profile_hook = lambda: _hook
            _mod.set_axon_ntff_profile_hook = lambda h: None
            sys.modules["antenv.axon_hooks"] = _mod
        import concourse.bass_utils as _bu
        _bu.upload_artifacts = lambda d: d  # no cloud copy in this container
        kwargs = dict(trace=True, trace_cores=[0])
        if _trace_dir is not None:
            kwargs["tmpdir"] = _trace_dir
    res = run_bass_kernel_spmd(nc, in_maps, core_ids=list(range(NCORES)),
                               **kwargs)

    out = np.empty((B, S, D), dtype=np.float32)
    bo32 = np.asarray(bo, np.float32)
    for b in range(B):
        acc = res.results[b * NGROUPS]["out_t"].astype(np.float32)
        for g in range(1, NGROUPS):
            acc = acc + res.results[b * NGROUPS + g]["out_t"].astype(
                np.float32)
        out[b] = acc.T + bo32
    if _profile:
        kernel._last_exec_time_ns = res.exec_time_ns
        kernel._last_results = res
    return out
